# revision 10
# baseline (speedup 1.0000x reference)
"""DreamAttention (GQA + RoPE + causal) on 8 trn2 NeuronCores.

Sharding: DP=2 over batch x sequence-parallel over q-tiles. Core c ->
(batch b = c // 4, seq rank r = c % 4). Core r owns q-tiles
[r, 7-r, 8+r, 15-r] (128 rows each, ascending) — every core gets exactly 34
k-tile-blocks of causal attention work, so the load is perfectly balanced.

K/V projection is sharded over the sequence: core r computes K/V only for
seq chunk r (512 positions, all 4 kv heads), applies K-RoPE and the V
transpose locally, then an AllGather over the 4-core batch group assembles
the full K/V on every core. The gather (~1MB out per core) overlaps with the
Q projection, which doesn't depend on it.

All matmul operands are bf16 (fp32 PSUM accumulation): same PE cycles/row as
fp32r for wide tiles, but no 4x penalty on narrow (<256) moving dims, FWL
halves LDWEIGHTS, and DMA/SBUF traffic halves.

Per-core dataflow:
  - projections: QT [d, 28h, 512q] (own q rows); KT/V for own seq chunk
  - RoPE via SBUF->SBUF DMA partition rotation + DVE mul/add
  - attention in transposed form per (head, k-tile): S^T[k, q-suffix] ->
    exp -> PV accumulates out^T[d, q]; the ascending q-tile layout makes the
    causally-live q columns a suffix, so each k-tile processes only [128, w]
    with w in {512, 384, 256, 128}; causality inside the suffix is enforced
    with a host-built additive mask. Same-width k-tiles are grouped so the
    mask add and the exp run on [128, g, w] batches (fewer, larger ScalarE /
    DVE ops); scores for group i+1 are emitted before PV of group i so the
    PE never waits on the exp. Softmax sums via ones-matmul into a PSUM
    accumulator; normalization fused into the PSUM->SBUF move, which
    overwrites the spent Q slice.
  - o_proj: attnT stationary, full Wo moving, accumulate over 28 head-chunks;
    output rows are core-owned -> DMA straight to the external output
Host reassembles the 8 cores' row-slices into the full [2, 2048, 3584] output.
"""

import math

import numpy as np
import ml_dtypes

import concourse.bass as bass
import concourse.mybir as mybir
import concourse.tile as tile
from concourse import bacc
from concourse.bass_utils import run_bass_kernel_spmd
from concourse.masks import make_identity

F32 = mybir.dt.float32
BF16 = mybir.dt.bfloat16
NP_BF16 = ml_dtypes.bfloat16

B, S, D = 2, 2048, 3584
H, KVH, HD = 28, 4, 128
ROPE_THETA = 1000000.0
GQ = H // KVH   # 7 q heads per kv head
DKT = D // 128  # 28 k-tiles over D
SC = 512        # seq chunk per core for K/V projection
NSC = S // SC   # 4
NKT = S // 128  # 16 k tiles over sequence
NDC = 7         # output D chunks of 512
NQT = 4         # q-tiles owned per core
QW = NQT * 128  # 512 q columns per core
SCALE = 1.0 / math.sqrt(HD)

# Same-width k-tile groups for the attention inner loop: (start, g, w).
# k-tiles start..start+g-1 all have live-suffix width w = _wof(start).
KT_GROUPS = [(0, 2, 512), (2, 2, 512), (4, 2, 384), (6, 2, 384),
             (8, 4, 256), (12, 4, 128)]


def _qtiles(r):
    """Ascending q-tile ids owned by seq-rank r; sum of (t+1) == 34 for all r."""
    return [r, 7 - r, 8 + r, 15 - r]


def _wof(kti):
    # Live-suffix width for k-tile kti. Rank-independent: every rank's
    # ascending tile list [t0<t1<t2<t3] satisfies t0<=3, 4<=t1<=7, 8<=t2<=11,
    # 12<=t3<=15, so #(tiles >= kti) == 4 - kti//4 for all ranks.
    return 128 * (4 - kti // 4)


_NC_CACHE = {}


def _build_nc(loop_n=1, phases="ABC"):
    key = ("nc", loop_n, phases)
    if key in _NC_CACHE:
        return _NC_CACHE[key]

    nc = bacc.Bacc("TRN2", target_bir_lowering=False, debug=False, num_devices=8)

    xq_d = nc.dram_tensor("xq", [DKT, 128, QW], BF16, kind="ExternalInput").ap()
    # x^T for the core's own seq chunk only (K/V proj is seq-sharded)
    xt_d = nc.dram_tensor("xt", [DKT, 128, SC], BF16, kind="ExternalInput").ap()
    wq_d = nc.dram_tensor("wq", [H, 128, DKT, 128], BF16, kind="ExternalInput").ap()
    wkv_d = nc.dram_tensor(
        "wkv", [2 * KVH, 2, 128, DKT // 2, 128], BF16, kind="ExternalInput"
    ).ap()
    wo_d = nc.dram_tensor("wo", [NDC, DKT, 128, 512], BF16, kind="ExternalInput").ap()
    cosq_d = nc.dram_tensor("cosq", [128, QW], BF16, kind="ExternalInput").ap()
    sinq_d = nc.dram_tensor("sinq", [128, QW], BF16, kind="ExternalInput").ap()
    cosk_d = nc.dram_tensor("cosk", [128, SC], BF16, kind="ExternalInput").ap()
    sink_d = nc.dram_tensor("sink", [128, SC], BF16, kind="ExternalInput").ap()
    mask_d = nc.dram_tensor("mask", [NKT, 128, 128], F32, kind="ExternalInput").ap()
    out_d = nc.dram_tensor("out", [NQT, 128, D], F32, kind="ExternalOutput").ap()

    with tile.TileContext(nc) as tc:
        with (
            tc.tile_pool(name="persist", bufs=1) as persist,
            tc.tile_pool(name="kvp", bufs=1) as kvp,
            tc.tile_pool(name="dramp", bufs=1, space="DRAM") as dramp,
        ):
            # qt doubles as the attention-output buffer: att(h) overwrites
            # qt[:, h, :] once head h's scores are done.
            qt = persist.tile([128, H, QW], BF16, name="qt")
            ident = persist.tile([128, 128], F32, name="ident")
            ones = persist.tile([128, 1], BF16, name="ones")
            ones_f = persist.tile([128, 1], F32, name="ones_f")

            # full K^T / V for attention, assembled by the AllGather
            kt_t = kvp.tile([128, KVH, NSC, SC], BF16, name="kt")
            vn = kvp.tile([128, KVH, NSC, SC], BF16, name="vn")

            make_identity(nc, ident)
            nc.vector.memset(ones_f, 1.0)
            nc.vector.tensor_copy(ones, ones_f)

            def rope(dst, cos_ap, sin_ap, width, tmp, dma_eng):
                t = tmp[:, :width]
                dma_eng.dma_start(out=t[0:64, :], in_=dst[64:128, :])
                dma_eng.dma_start(out=t[64:128, :], in_=dst[0:64, :])
                nc.vector.tensor_mul(t, t, sin_ap)
                nc.vector.tensor_mul(dst, dst, cos_ap)
                nc.vector.tensor_add(dst, dst, t)

            for _rep in range(loop_n):
                # opened before B so the wo DMAs prefetch during attention
                wop_cm = tc.tile_pool(name="wop", bufs=56)
                wop = wop_cm.__enter__()
                outp_cm = tc.tile_pool(name="outp", bufs=3)
                outp = outp_cm.__enter__()
                if "A" in phases:
                    with tc.tile_pool(name="ps_proj", bufs=2, space="PSUM") as ps_proj:
                        # ---- Phase A2': K/V projection for own seq chunk ----
                        with (
                            tc.tile_pool(name="xtp", bufs=28) as xtp,
                            tc.tile_pool(name="wkvp", bufs=4) as wkvp,
                            tc.tile_pool(name="kvcp", bufs=1) as kvcp,
                            tc.tile_pool(name="ktab", bufs=1) as ktab,
                            tc.tile_pool(name="kvtmp", bufs=2) as kvtmp,
                        ):
                            coskc = ktab.tile([128, SC], BF16, name="coskc")
                            sinkc = ktab.tile([128, SC], BF16, name="sinkc")
                            nc.gpsimd.dma_start(out=coskc, in_=cosk_d)
                            nc.gpsimd.dma_start(out=sinkc, in_=sink_d)
                            xts = []
                            for kti in range(DKT):
                                xtile = xtp.tile([128, SC], BF16, name="xt")
                                nc.sync.dma_start(out=xtile, in_=xt_d[kti])
                                xts.append(xtile)
                            kt_c = kvcp.tile([128, KVH, SC], BF16, name="kt_c")
                            vtc = kvcp.tile([128, KVH, SC], F32, name="vtc")
                            vnc = kvcp.tile([128, KVH, SC], BF16, name="vnc")
                            for ct in range(2 * KVH):  # 0-3: K heads, 4-7: V
                                psum = ps_proj.tile([128, SC], F32, name="pp")
                                for hf in range(2):
                                    wblk = wkvp.tile(
                                        [128, DKT // 2, 128], BF16, name="wkv"
                                    )
                                    # wkv on scalar so the first matmul
                                    # isn't stuck behind the 28 xt DMAs on
                                    # sync
                                    nc.scalar.dma_start(out=wblk, in_=wkv_d[ct, hf])
                                    for kti in range(DKT // 2):
                                        gkt = hf * (DKT // 2) + kti
                                        nc.tensor.matmul(
                                            psum,
                                            wblk[:, kti, :],
                                            xts[gkt],
                                            start=(gkt == 0),
                                            stop=(gkt == DKT - 1),
                                        )
                                if ct < KVH:
                                    nc.vector.tensor_copy(kt_c[:, ct, :], psum)
                                else:
                                    nc.vector.tensor_copy(vtc[:, ct - KVH, :], psum)
                            # V^T -> V natural (16 x 128x128 PE transposes)
                            for kv in range(KVH):
                                for sti in range(4):
                                    ptr = ps_proj.tile([128, 128], F32, name="ptr")
                                    nc.tensor.transpose(
                                        ptr,
                                        vtc[:, kv, sti * 128 : (sti + 1) * 128],
                                        ident,
                                    )
                                    nc.vector.tensor_copy(
                                        vnc[:, kv, sti * 128 : (sti + 1) * 128], ptr
                                    )
                            # K-RoPE on own chunk
                            for kv in range(KVH):
                                tmp = kvtmp.tile([128, SC], BF16, name="ropetmp")
                                rope(
                                    kt_c[:, kv, :], coskc, sinkc, SC, tmp, nc.gpsimd
                                )
                            # ---- AllGather K/V across the 4-core batch group
                            inb = dramp.tile([128, 2, KVH, SC], BF16, name="inb")
                            outb = dramp.tile(
                                [NSC, 128, 2, KVH, SC], BF16, name="outb"
                            )
                            nc.gpsimd.dma_start(out=inb[:, 0], in_=kt_c)
                            nc.gpsimd.dma_start(out=inb[:, 1], in_=vnc)
                            nc.gpsimd.collective_compute(
                                "AllGather",
                                mybir.AluOpType.bypass,
                                replica_groups=[[0, 1, 2, 3], [4, 5, 6, 7]],
                                ins=[inb.opt()],
                                outs=[outb.opt()],
                            )
                            # bring the gathered full K/V into SBUF (gpsimd
                            # queue: sits behind the collective, off the
                            # critical A1 DMA queues)
                            for c in range(NSC):
                                nc.gpsimd.dma_start(
                                    out=kt_t[:, :, c, :], in_=outb[c, :, 0]
                                )
                                nc.gpsimd.dma_start(
                                    out=vn[:, :, c, :], in_=outb[c, :, 1]
                                )

                        # ---- Phase A1: Q projection + fused Q-RoPE ----
                        with (
                            tc.tile_pool(name="xqp", bufs=1) as xqp,
                            tc.tile_pool(name="wqp", bufs=3) as wqp,
                            tc.tile_pool(name="qtab", bufs=1) as qtab,
                            tc.tile_pool(name="qrtmp", bufs=3) as qrtmp,
                        ):
                            cosq = qtab.tile([128, QW], BF16, name="cosq")
                            sinq = qtab.tile([128, QW], BF16, name="sinq")
                            nc.scalar.dma_start(out=cosq, in_=cosq_d)
                            nc.scalar.dma_start(out=sinq, in_=sinq_d)
                            xq = xqp.tile([128, DKT, QW], BF16, name="xq")
                            nc.scalar.dma_start(
                                out=xq, in_=xq_d.rearrange("k p q -> p k q")
                            )
                            for ct in range(H):
                                wblk = wqp.tile([128, DKT, 128], BF16, name="wq")
                                nc.sync.dma_start(out=wblk, in_=wq_d[ct])
                                psum = ps_proj.tile([128, QW], F32, name="pp")
                                for kti in range(DKT):
                                    nc.tensor.matmul(
                                        psum,
                                        wblk[:, kti, :],
                                        xq[:, kti, :],
                                        start=(kti == 0),
                                        stop=(kti == DKT - 1),
                                    )
                                nc.vector.tensor_copy(qt[:, ct, :], psum)
                                tmp = qrtmp.tile([128, QW], BF16, name="qrtmp")
                                rope(qt[:, ct, :], cosq, sinq, QW, tmp, nc.scalar)

                if "B" in phases:
                    # ---- Phase B: attention, 28 heads on the core's 512 q ----
                    with (
                        tc.tile_pool(name="ps_sg", bufs=2, space="PSUM") as ps_sg,
                        tc.tile_pool(name="ps_o", bufs=2, space="PSUM") as ps_o,
                        tc.tile_pool(name="ps_sum", bufs=2, space="PSUM") as ps_sum,
                        tc.tile_pool(name="ptp", bufs=3) as ptp,
                        tc.tile_pool(name="accp", bufs=2) as accp,
                        tc.tile_pool(name="smallp", bufs=2) as smallp,
                        tc.tile_pool(name="maskp", bufs=1) as maskp,
                    ):
                        mask_t = maskp.tile([128, NKT, 128], F32, name="mask_t")
                        nc.sync.dma_start(
                            out=mask_t, in_=mask_d.rearrange("k p q -> p k q")
                        )
                        norm_pending = None

                        def normalize(h, psum_o, psum_r):
                            rec = smallp.tile([1, QW], F32, name="rec")
                            nc.vector.reciprocal_approx_fast(rec, psum_r)
                            bcast = smallp.tile([128, QW], F32, name="bcast")
                            nc.gpsimd.partition_broadcast(bcast, rec)
                            # fused normalize + PSUM->SBUF, overwriting head
                            # h's spent q columns
                            nc.vector.tensor_mul(qt[:, h, :], psum_o, bcast)

                        for h in range(H):
                            kv = h // GQ
                            psum_o = ps_o.tile([128, QW], F32, name="po")
                            psum_r = ps_sum.tile([1, QW], F32, name="psr")
                            # row-sum accumulator for the narrow k-tiles
                            # (4..15); tiles 0..3 are summed by PE
                            # ones-matmuls directly. acc[:, 0:128] is never
                            # live (q < 128 only attends k-tiles 0..3).
                            acc = accp.tile([128, QW], BF16, name="acc")

                            def emit_pv(grp, pt):
                                start, g, w = grp
                                lo = QW - w
                                for j in range(g):
                                    kti = start + j
                                    nc.tensor.matmul(
                                        psum_o[:, lo:],
                                        vn[
                                            :, kv, kti // 4,
                                            (kti % 4) * 128 : (kti % 4 + 1) * 128,
                                        ],
                                        pt[:, j, 0:w],
                                        start=(kti == 0),
                                        stop=(kti == NKT - 1),
                                    )
                                if start == 0:  # tiles 0,1: PE row-sums
                                    for j in range(g):
                                        nc.tensor.matmul(
                                            psum_r,
                                            ones,
                                            pt[:, j, :],
                                            start=(j == 0),
                                            stop=False,
                                        )
                                elif start == 2:  # tiles 2,3: PE row-sums
                                    for j in range(g):
                                        nc.tensor.matmul(
                                            psum_r,
                                            ones,
                                            pt[:, j, :],
                                            start=False,
                                            stop=False,
                                        )
                                elif start == 4:
                                    # first narrow group: acc = pt0 + pt1
                                    nc.vector.tensor_add(
                                        acc[:, lo:], pt[:, 0, 0:w], pt[:, 1, 0:w]
                                    )
                                else:
                                    for j in range(g):
                                        nc.vector.tensor_add(
                                            acc[:, lo:], acc[:, lo:], pt[:, j, 0:w]
                                        )

                            prev = None
                            for grp in KT_GROUPS:
                                start, g, w = grp
                                lo = QW - w
                                # stride-pad w=384 groups to 512 so no matmul
                                # output crosses a PSUM bank boundary
                                wp = 512 if w > 256 else w
                                psum_s = ps_sg.tile([128, g, wp], F32, name="pss")
                                for j in range(g):
                                    kti = start + j
                                    nc.tensor.matmul(
                                        psum_s[:, j, 0:w],
                                        kt_t[
                                            :, kv, kti // 4,
                                            (kti % 4) * 128 : (kti % 4 + 1) * 128,
                                        ],
                                        qt[:, h, lo:],
                                        start=True,
                                        stop=True,
                                    )
                                # batched diagonal-mask add + batched exp
                                nc.vector.tensor_add(
                                    psum_s[:, :, 0:128],
                                    psum_s[:, :, 0:128],
                                    mask_t[:, start : start + g, :],
                                )
                                pt = ptp.tile([128, g, wp], BF16, name="pt")
                                nc.scalar.activation(
                                    pt[:, :, 0:w],
                                    psum_s[:, :, 0:w],
                                    mybir.ActivationFunctionType.Exp,
                                )
                                if prev is not None:
                                    emit_pv(*prev)
                                prev = (grp, pt)
                            emit_pv(*prev)
                            # fold the DVE-accumulated narrow tiles into the
                            # row sums (single 384-wide ones-matmul)
                            nc.tensor.matmul(
                                psum_r[:, 128:],
                                ones,
                                acc[:, 128:],
                                start=False,
                                stop=True,
                            )
                            if norm_pending is not None:
                                normalize(*norm_pending)
                            norm_pending = (h, psum_o, psum_r)
                        normalize(*norm_pending)

                if "C" in phases:
                    # ---- Phase C: o_proj (full Wo); rows are core-owned ----
                    with (
                        tc.tile_pool(name="ps_c", bufs=2, space="PSUM") as ps_c,
                    ):
                        for dc in range(NDC):
                            wo_tiles = []
                            for ct in range(DKT):
                                wt = wop.tile([128, 512], BF16, name="wo")
                                if ct % 2 == 0:
                                    nc.sync.dma_start(out=wt, in_=wo_d[dc, ct])
                                else:
                                    nc.gpsimd.dma_start(out=wt, in_=wo_d[dc, ct])
                                wo_tiles.append(wt)
                            for q in range(NQT):
                                psum = ps_c.tile([128, 512], F32, name="pp")
                                for ct in range(DKT):
                                    nc.tensor.matmul(
                                        psum,
                                        qt[:, ct, q * 128 : (q + 1) * 128],
                                        wo_tiles[ct],
                                        start=(ct == 0),
                                        stop=(ct == DKT - 1),
                                    )
                                ob = outp.tile([128, 512], F32, name="ob")
                                nc.vector.tensor_copy(ob, psum)
                                nc.sync.dma_start(
                                    out=out_d[q, :, dc * 512 : (dc + 1) * 512],
                                    in_=ob,
                                )
                outp_cm.__exit__(None, None, None)
                wop_cm.__exit__(None, None, None)

    nc.finalize()
    _NC_CACHE[key] = nc
    return nc


def _host_inputs(hidden_states, Wq, Wk, Wv, Wo):
    hidden = np.asarray(hidden_states, dtype=np.float32)
    Wq = np.asarray(Wq, dtype=np.float32) * np.float32(SCALE)
    Wk = np.asarray(Wk, dtype=np.float32)
    Wv = np.asarray(Wv, dtype=np.float32)
    Wo = np.asarray(Wo, dtype=np.float32)

    inv_freq = 1.0 / ROPE_THETA ** (np.arange(0, HD, 2, dtype=np.float32) / HD)
    t = np.arange(S, dtype=np.float32)
    freqs = np.outer(t, inv_freq)  # [S, 64]
    cos_t = np.cos(freqs.T)  # [64, S]
    sin_t = np.sin(freqs.T)
    cosk = np.concatenate([cos_t, cos_t], axis=0).astype(np.float32)  # [128, S]
    sink = np.concatenate([-sin_t, sin_t], axis=0).astype(np.float32)

    # shared weight layouts (identical for every core)
    wq = np.ascontiguousarray(
        Wq.reshape(DKT, 128, H, 128).transpose(2, 1, 0, 3)
    ).astype(NP_BF16)  # [h, p, kt, c]
    wk4 = Wk.reshape(DKT, 128, KVH, 128)
    wv4 = Wv.reshape(DKT, 128, KVH, 128)
    wkv = np.empty((2 * KVH, 2, 128, DKT // 2, 128), np.float32)
    for ct in range(KVH):
        for hf in range(2):
            ktsl = slice(hf * (DKT // 2), (hf + 1) * (DKT // 2))
            wkv[ct, hf] = wk4[ktsl, :, ct, :].transpose(1, 0, 2)
            wkv[KVH + ct, hf] = wv4[ktsl, :, ct, :].transpose(1, 0, 2)
    wkv = wkv.astype(NP_BF16)
    wo = np.ascontiguousarray(
        Wo.reshape(DKT, 128, NDC, 512).transpose(2, 0, 1, 3)
    ).astype(NP_BF16)  # [dc, ct, p, d]

    in_maps = []
    for core in range(8):
        b, r = core // 4, core % 4
        tiles = _qtiles(r)
        qpos = np.concatenate(
            [np.arange(t0 * 128, (t0 + 1) * 128) for t0 in tiles]
        )  # [512] ascending global q positions
        xq = np.ascontiguousarray(
            hidden[b][qpos].reshape(QW, DKT, 128).transpose(1, 2, 0)
        ).astype(NP_BF16)  # [kt, p, q]
        csl = slice(r * SC, (r + 1) * SC)
        xt = np.ascontiguousarray(
            hidden[b][csl].reshape(SC, DKT, 128).transpose(1, 2, 0)
        ).astype(NP_BF16)  # [kt, p, s] for own chunk
        cosq = np.ascontiguousarray(cosk[:, qpos]).astype(NP_BF16)
        sinq = np.ascontiguousarray(sink[:, qpos]).astype(NP_BF16)
        # mask[kt]: [128, 128] additive mask for the FIRST live block of the
        # suffix (columns QW-w .. QW-w+128). Triangular when that block's
        # q-tile equals kt (the diagonal), all-zero otherwise.
        mask = np.zeros((NKT, 128, 128), np.float32)
        for kti in range(NKT):
            lo = QW - _wof(kti)
            kk = kti * 128 + np.arange(128)[:, None]
            qq = qpos[None, lo : lo + 128]
            mask[kti] = np.where(kk <= qq, 0.0, -30000.0)
        in_maps.append(
            {
                "xq": xq,
                "xt": xt,
                "wq": wq,
                "wkv": wkv,
                "wo": wo,
                "cosq": cosq,
                "sinq": sinq,
                "cosk": np.ascontiguousarray(cosk[:, csl]).astype(NP_BF16),
                "sink": np.ascontiguousarray(sink[:, csl]).astype(NP_BF16),
                "mask": mask,
            }
        )
    return in_maps


def kernel(hidden_states, Wq, Wk, Wv, Wo, trace=False):
    nc = _build_nc()
    in_maps = _host_inputs(hidden_states, Wq, Wk, Wv, Wo)
    res = run_bass_kernel_spmd(nc, in_maps, list(range(8)), trace=trace)
    out = np.empty((B, S, D), dtype=np.float32)
    for core in range(8):
        b, r = core // 4, core % 4
        o = res.results[core]["out"]  # [NQT, 128, D]
        for j, t0 in enumerate(_qtiles(r)):
            out[b, t0 * 128 : (t0 + 1) * 128, :] = o[j]
    if trace:
        kernel.last_exec_time_ns = res.exec_time_ns
    return out
